# revision 14
# baseline (speedup 1.0000x reference)
"""Trainium2 Bass kernel for the non-local attention denoising block.

Computation (per batch b of the [2, 3, 96, 96] input):
    x      = input[b].reshape(3, 9216)                  # [C, N]
    S      = x^T x / sqrt(3)                            # [N, N] never materialized
    A      = softmax(S, axis=1)
    f      = (A @ x^T)^T                                # [C, N]
    out[b] = input[b] + conv3x3_same(f) + conv_b

Sharding: 8 cores = 2 batches x 4 query bands of 24 image rows. Each core
computes EXACTLY its 2304 band queries against the full key sequence,
flash-attention style; the two F halo rows the 3x3 conv needs come from
the neighboring cores via a tiny (2.3KB) AllGather over the 4 cores of
each batch, instead of being recomputed (recomputation costs 8.3% more
exp work on the scalar engine, which is the kernel bottleneck: ~155us of
ACTIVATE at 128 lanes / 1.2 GHz is the floor for 2304x9216 exps).

  - Query columns are host-permuted to [row0, row23, row1..row22] so the
    band's edge rows finish in the first (256-query) chunk; their divided
    F values are exchanged early and the collective hides in slack.
  - SPMD cores share one NEFF, so "which gather entries are my
    neighbors" is data, not code: host-built per-core 0/1 masks select
    (and zero, at the image boundary) the gathered edge rows via a
    DVE multiply + reduce.
  - mm1: S^T tile [128 k-part, q-free] = matmul(lhsT=xk block, rhs=xq).
    The C=3 contraction is zero-padded to K=128 on-chip (memset + 3-row
    DMA), in per-piece tiles because the Tile framework tracks
    dependencies at tile granularity -- one big tile would stall the
    first matmul on the last memset.
  - exp on the scalar engine, 3 k-blocks per instruction (free dim
    KG*qn <= 1536 in a 6KB PSUM slot, double buffered; U accumulator
    and transpose slots take the last 2 of 8 PSUM banks).
  - mm2: U^T[4, q] += matmul(lhsT=v4[kb] ([128, 4] = x^T with a ones
    column accumulating the softmax denominator Z), rhs=e^T), spread
    over three PE column groups so drains proceed concurrently.
  - softmax divide: PE-transpose the column-group partials into [128,4],
    DVE reciprocal + tensor_scalar multiply, PE-transpose back; emitted
    per 128-query piece, paced one piece per few key-groups inside the
    next chunk so the PE work rides the per-group slack.
  - conv: F lands in a width-98 zero-padded row layout, 9 shifted
    SBUF->SBUF DMAs build a [27, 2352] stack, the 3x3 conv is K=27
    matmuls; all emitted incrementally per finished F piece so the tail
    after the last exp is one piece + one small conv chain.

Query chunks are uniform KG=3 (small tail chunks with inflated KG are
LDWEIGHTS-bound on the PE and starve the scalar engine). No [N, N]
tensor ever touches HBM; per-core HBM traffic is ~0.3 MB.
"""

import math
import os
import sys

for _p in (
    "/opt/trn_rl_repo",
    "/root/.axon_site",
    "/root/.axon_site/_ro/trn_rl_repo",
    "/root/.axon_site/_ro/pypackages",
):
    if os.path.isdir(_p) and _p not in sys.path:
        sys.path.append(_p)

import ml_dtypes  # noqa: E402
import numpy as np  # noqa: E402

import concourse.bacc as bacc  # noqa: E402
import concourse.bass as bass  # noqa: E402
import concourse.tile as tile  # noqa: E402
from concourse import mybir  # noqa: E402
from concourse.bass_utils import run_bass_kernel_spmd  # noqa: E402

# Problem shape (hardcoded per the harness contract).
B, C, H, W = 2, 3, 96, 96
N = H * W                      # 9216 spatial positions (keys)
BANDS = 4                      # query bands per batch
BAND_ROWS = H // BANDS         # 24 image rows per band
QN = BAND_ROWS * W             # 2304 queries per core (no halo)
KB = N // 128                  # 72 key blocks of 128
KG = 3                         # k-blocks fused per exp instruction
NG = KB // KG                  # 24 key groups per chunk
WP = W + 2                     # padded image row width for the conv
CONVN = BAND_ROWS * WP         # 2352 conv output positions (padded layout)
FP_ROWS = BAND_ROWS + 2        # F_p rows incl one exchanged row each side
FPN = FP_ROWS * WP + 4         # F_p length (+4 slack for shift reads)
INV_SQRT_C = 1.0 / math.sqrt(C)

F32 = mybir.dt.float32
BF16 = mybir.dt.bfloat16
EXP = mybir.ActivationFunctionType.Exp

SHIFTS = [(dy, dx) for dy in (-1, 0, 1) for dx in (-1, 0, 1)]

# Query chunks (host-permuted column order [row0, row23, row1..row22]).
CHUNKS = [(0, 256), (256, 512), (768, 512), (1280, 512), (1792, 512)]
# xk SBUF pieces (kb edges 0, 6, 24, 72) so early matmuls only wait on
# the first small memset+DMA; later pieces' memsets hide under compute.
XK_EDGES = [0, 768, 3072, 9216]


def build_nc() -> bass.Bass:
    nc = bacc.Bacc(num_devices=8)

    xk = nc.declare_dram_parameter("xk", [C, N], BF16, isOutput=False)
    xq = nc.declare_dram_parameter("xq", [C, QN], BF16, isOutput=False)
    v4 = nc.declare_dram_parameter("v4", [128, KB * 4], BF16, isOutput=False)
    resid = nc.declare_dram_parameter("resid", [C, CONVN], F32, isOutput=False)
    msel = nc.declare_dram_parameter("msel", [C, 1536], F32, isOutput=False)
    wmat = nc.declare_dram_parameter("wmat", [27, C], BF16, isOutput=False)
    eye4x = nc.declare_dram_parameter("eye4x", [128, 4], F32, isOutput=False)
    id128 = nc.declare_dram_parameter("id128", [128, 128], F32, isOutput=False)
    out = nc.declare_dram_parameter("out", [C, QN], F32, isOutput=True)

    with tile.TileContext(nc) as tc, \
            tc.tile_pool(name="persist", bufs=1) as P, \
            tc.tile_pool(name="work", bufs=4) as WK, \
            tc.tile_pool(name="small", bufs=8) as SM, \
            tc.tile_pool(name="epool", bufs=8) as EPl, \
            tc.tile_pool(name="dram", bufs=1, space="DRAM") as DR, \
            tc.tile_pool(name="spool", bufs=2, space="PSUM") as SP, \
            tc.tile_pool(name="upool", bufs=1, space="PSUM") as UP, \
            tc.tile_pool(name="tpool", bufs=1, space="PSUM") as TP:

        # ---- load inputs -------------------------------------------------
        # xk/xq are [3, *] in HBM; the K=128 zero padding happens on-chip
        # (both operand pads must be deterministic zeros: SBUF garbage can
        # hold Inf/NaN bit patterns and 0*Inf = NaN). Piecewise tiles keep
        # the first matmul off the critical path of the later memsets, and
        # the memset order matches first-use order (DVE queue is in-order).
        xq0_sb = P.tile([128, CHUNKS[0][1]], BF16, tag="xq0", name="xq0_sb")
        v4_sb = P.tile([128, KB * 4], BF16, tag="v4", name="v4_sb")
        resid_sb = P.tile([C, CONVN], F32, tag="resid", name="resid_sb")
        msel_sb = P.tile([C, 1536], F32, tag="msel", name="msel_sb")
        wmat_sb = P.tile([27, C], BF16, tag="wmat", name="wmat_sb")
        eye4x_sb = P.tile([128, 4], F32, tag="eye4x", name="eye4x_sb")
        id128_sb = P.tile([128, 128], F32, tag="id128", name="id128_sb")
        QR = QN - CHUNKS[0][1]       # chunks 1..4 share one xq tile
        xqr_sb = P.tile([128, QR], BF16, tag="xqr", name="xqr_sb")

        # memset order matches first-use order (DVE queue is in-order),
        # and the no-dependency v4 DMA goes ahead of the xqr DMA on the
        # sync queue so the last memset can't head-of-line block it.
        nc.vector.memset(xq0_sb, 0.0)
        nc.sync.dma_start(out=xq0_sb[0:C, :], in_=xq[:, 0:CHUNKS[0][1]])
        nc.sync.dma_start(out=v4_sb, in_=v4[:])
        xk_t = []
        for i, (a, b_) in enumerate(zip(XK_EDGES, XK_EDGES[1:])):
            t = P.tile([128, b_ - a], BF16, tag=f"xk{i}", name=f"xk{i}_sb")
            nc.vector.memset(t, 0.0)
            nc.gpsimd.dma_start(out=t[0:C, :], in_=xk[:, a:b_])
            xk_t.append(t)
        nc.sync.dma_start(out=resid_sb, in_=resid[:])
        nc.sync.dma_start(out=msel_sb, in_=msel[:])
        nc.gpsimd.dma_start(out=wmat_sb, in_=wmat[:])
        nc.gpsimd.dma_start(out=eye4x_sb, in_=eye4x[:])
        nc.gpsimd.dma_start(out=id128_sb, in_=id128[:])
        nc.vector.memset(xqr_sb, 0.0)
        nc.sync.dma_start(out=xqr_sb[0:C, :], in_=xq[:, CHUNKS[0][1]:QN])

        def xq_chunk(ci):
            q0, qn = CHUNKS[ci]
            if ci == 0:
                return xq0_sb[:, 0:qn]
            return xqr_sb[:, q0 - CHUNKS[0][1]:q0 - CHUNKS[0][1] + qn]

        def xk_block(kb):
            c0 = kb * 128
            for i, (a, b_) in enumerate(zip(XK_EDGES, XK_EDGES[1:])):
                if a <= c0 < b_:
                    return xk_t[i][:, c0 - a:c0 - a + 128]
            raise AssertionError(kb)

        F_sb = P.tile([C, QN], F32, tag="F", name="F_sb")
        F_p = P.tile([C, FPN], BF16, tag="Fp", name="F_p")
        nc.gpsimd.memset(F_p, 0.0)
        F_sh = P.tile([27, CONVN], BF16, tag="Fsh", name="F_sh")
        nc.gpsimd.memset(F_sh, 0.0)
        out_pad = P.tile([C, CONVN], F32, tag="opad", name="out_pad")
        gx_sb = P.tile([C, 768], F32, tag="gx", name="gx_sb")
        F_p_r = F_p[:, 0:FP_ROWS * WP].rearrange("p (r w) -> p r w", w=WP)
        cc_in = DR.tile([C, 192], F32, tag="ccin", name="cc_in")
        cc_out = DR.tile([4 * C, 192], F32, tag="ccout", name="cc_out")
        pstate = {"q": 0, "cols": 98, "rc": 0, "out": 1}
        # The conv window [98, 2253) only reads F_p rows 1..24 plus the
        # always-zero pad columns of rows 0/25 (reads for col c >= 98 hit
        # row 0 only at its col-97 zero, and row 25 only at its col-0
        # zero), so the whole interior runs before the exchange blend
        # lands; the head/tail conv rows (cols [0,98) and [2253,2352))
        # are emitted in finalize() after it.
        INT_EDGES = [98, 490, 979, 1469, 1959, 2253]
        out_pad_r = out_pad.rearrange("p (r w) -> p r w", w=WP)
        out_r = out[:].rearrange("p (r w) -> p r w", w=W)

        # F column q -> F_p position: cols [0,96) = image row 0 (F_p row
        # 1), [96,192) = image row 23 (F_p row 24), 192+k = image row
        # 1+k//96 (F_p row 2+k//96). F_p rows 0/25 come from the exchange.
        def fp_copy(lo, hi):
            def seg(src_lo, src_hi, row, col):
                nc.vector.tensor_copy(
                    F_p_r[:, row, col:col + src_hi - src_lo],
                    F_sb[:, src_lo:src_hi])
            if lo < 96:
                seg(lo, min(hi, 96), 1, 1 + lo)
            if lo < 192 and hi > 96:
                a = max(lo, 96)
                seg(a, min(hi, 192), 24, 1 + a - 96)
            i0, i1 = max(lo, 192) - 192, hi - 192
            if i1 > i0:
                if i0 % 96:
                    h = min(i1, (i0 // 96 + 1) * 96)
                    seg(192 + i0, 192 + h, 2 + i0 // 96, 1 + i0 % 96)
                    i0 = h
                nr = (i1 - i0) // 96
                if nr > 0:
                    nc.vector.tensor_copy(
                        F_p_r[:, 2 + i0 // 96:2 + i0 // 96 + nr, 1:1 + W],
                        F_sb[:, 192 + i0:192 + i0 + 96 * nr].rearrange(
                            "p (r w) -> p r w", w=W))
                    i0 += 96 * nr
                if i1 > i0:
                    seg(192 + i0, 192 + i1, 2 + i0 // 96, 1)

        def emit_shifts(lo, hi, engs):
            for s, (dy, dx) in enumerate(SHIFTS):
                off = WP + dy * WP + dx
                a = max(lo, -off)
                if hi > a:
                    engs[s % len(engs)].dma_start(
                        out=F_sh[3 * s:3 * s + 3, a:hi],
                        in_=F_p[:, a + off:hi + off])

        def emit_conv(c0, c1):
            cv_ps = TP.tile([C, c1 - c0], F32, tag="T", name="cv_ps",
                            padded_shape=[C, 512])
            nc.tensor.matmul(cv_ps, lhsT=wmat_sb, rhs=F_sh[:, c0:c1],
                             start=True, stop=True)
            nc.vector.tensor_add(out_pad[:, c0:c1], cv_ps,
                                 resid_sb[:, c0:c1])

        def rows_ready():
            # contiguous F_p rows from row 1, given q queries divided
            q = pstate["q"]
            for r in range(1, 25):
                if r == 1:
                    ok = q >= 96
                elif r == 24:
                    ok = q >= 192
                else:
                    ok = q >= 288 + 96 * (r - 2)
                if not ok:
                    return r
            return 25

        def advance():
            # Shifts are emitted in big batches (per-DMA SWDGE issue cost
            # is ~0.6us) and lead the conv pieces by at least one
            # epilogue-pop interval, so a conv matmul never head-of-line
            # blocks the PE queue waiting on a just-issued shift DMA.
            lim = min(2253, rows_ready() * WP - 197)
            prev = pstate["cols"]
            if lim - prev >= 392 or (lim >= 2253 and prev < 2253):
                emit_shifts(prev, lim, [nc.gpsimd, nc.sync])
                pstate["cols"] = lim
            while pstate["rc"] + 1 < len(INT_EDGES) and \
                    INT_EDGES[pstate["rc"] + 1] <= prev:
                c0 = INT_EDGES[pstate["rc"]]
                c1 = INT_EDGES[pstate["rc"] + 1]
                emit_conv(c0, c1)
                pstate["rc"] += 1
                done = min(c1 // WP, 22)
                if done - pstate["out"] >= 5:
                    nc.sync.dma_start(
                        out=out_r[:, pstate["out"]:done, :],
                        in_=out_pad_r[:, pstate["out"]:done, 1:1 + W])
                    pstate["out"] = done

        def exchange_start():
            # edge F rows (divided, in-image) -> HBM -> AllGather over the
            # 4 cores of this batch -> back to SBUF. Issued early (during
            # chunk 1) so the collective latency and cross-core skew hide
            # behind the attention main loop.
            nc.sync.dma_start(out=cc_in[:], in_=F_sb[:, 0:192])
            nc.gpsimd.collective_compute(
                "AllGather",
                mybir.AluOpType.bypass,
                replica_groups=[[0, 1, 2, 3], [4, 5, 6, 7]],
                ins=[cc_in[:].opt()],
                outs=[cc_out[:].opt()],
            )
            nc.sync.dma_start(
                out=gx_sb.rearrange("p (g n) -> p g n", n=192),
                in_=cc_out[:].rearrange("(g c) n -> c g n", c=C))

        def exchange_blend():
            # per-core masked blend of the gathered edge rows into the F_p
            # halo rows. Out-of-image edges have all-zero masks, so the
            # blend also implements the conv's zero padding. Deferred to
            # chunk 3 (data long arrived): the DVE queue is in-order, so
            # emitting this any earlier would stall every later DVE op on
            # the collective.
            for side, row in ((0, 0), (1, FP_ROWS - 1)):
                t_bl = SM.tile([C, 768], F32, tag="bl", name="t_bl")
                nc.vector.tensor_mul(
                    t_bl, gx_sb, msel_sb[:, side * 768:side * 768 + 768])
                r_bl = SM.tile([C, W], F32, tag="rbl", name="r_bl")
                nc.vector.tensor_reduce(
                    r_bl, t_bl.rearrange("p (g w) -> p w g", w=W),
                    axis=mybir.AxisListType.X, op=mybir.AluOpType.add)
                nc.vector.tensor_copy(F_p_r[:, row, 1:1 + W], r_bl)

        def finalize():
            # everything that needs the exchanged halo rows or the last
            # interior row: remaining interior conv, then head/tail rows.
            # No DMAs ever go on the scalar queue: the scheduler can place
            # them at earlier queue slots, and a not-yet-satisfiable F_p
            # dependency would then head-of-line block the ACTIVATE stream.
            advance()
            while pstate["rc"] + 1 < len(INT_EDGES):
                c0 = INT_EDGES[pstate["rc"]]
                c1 = INT_EDGES[pstate["rc"] + 1]
                emit_conv(c0, c1)
                pstate["rc"] += 1
            engs = [nc.gpsimd, nc.sync]
            emit_shifts(0, 98, engs)
            emit_shifts(2253, CONVN, engs)
            emit_conv(0, 98)
            emit_conv(2253, CONVN)
            nc.sync.dma_start(out=out_r[:, 0:1, :],
                              in_=out_pad_r[:, 0:1, 1:1 + W])
            nc.gpsimd.dma_start(
                out=out_r[:, pstate["out"]:BAND_ROWS, :],
                in_=out_pad_r[:, pstate["out"]:BAND_ROWS, 1:1 + W])

        # ---- flash attention main loop ----------------------------------
        pending_epi = []

        def emit_mm1(ci, t):
            q0, qn = CHUNKS[ci]
            S_ps = SP.tile([128, KG, qn], F32, tag="S", name="S_ps",
                           padded_shape=[128, KG, 512])
            for j in range(KG):
                kb = t * KG + j
                nc.tensor.matmul(
                    S_ps[:, j, :],
                    lhsT=xk_block(kb),
                    rhs=xq_chunk(ci),
                    start=True, stop=True,
                )
            return S_ps

        s_pre = {}
        for ci, (q0, qn) in enumerate(CHUNKS):
            ncg = 3                  # mm2/U PSUM column groups (bases 0/32/64)
            U_ps = UP.tile([128, qn], F32, tag="U", name="U_ps",
                           padded_shape=[128, 512])
            for t in range(NG):
                S_ps = s_pre.pop((ci, t), None)
                if S_ps is None:
                    S_ps = emit_mm1(ci, t)
                e_sb = EPl.tile([128, KG, qn], BF16, tag="e", name="e_sb",
                                padded_shape=[128, KG, 512])
                # epilogue pieces spread one per few groups: their PE
                # transposes fit in the per-group PE slack instead of
                # bursting between two groups and starving the scalar engine
                if pending_epi and t >= 3 and t % 3 == 0:
                    pending_epi.pop(0)()
                nc.scalar.activation(e_sb, S_ps, EXP, scale=INV_SQRT_C)
                if t == NG - 1 and ci + 1 < len(CHUNKS):
                    # pre-issue the next chunk's first two mm1 groups ahead
                    # of this chunk's last mm2 + U_sb copy, so the boundary
                    # U-bank release never starves the scalar engine
                    s_pre[(ci + 1, 0)] = emit_mm1(ci + 1, 0)
                    s_pre[(ci + 1, 1)] = emit_mm1(ci + 1, 1)
                for j in range(KG):
                    kb = t * KG + j
                    jj = j % ncg
                    nc.tensor.matmul(
                        U_ps[32 * jj:32 * jj + 4, 0:qn],
                        lhsT=v4_sb[:, kb * 4:kb * 4 + 4],
                        rhs=e_sb[:, j, :],
                        start=(t == 0 and j < ncg),
                        stop=(t == NG - 1 and j >= KG - ncg),
                        skip_group_check=True,
                    )

            # softmax division via PE transposes (partition-aligned only).
            # The U_sb copy stays here (it releases the single U_ps bank);
            # the PE transposes + divide are deferred into the next chunk's
            # group loop so boundary mm1s keep the scalar engine fed.
            U_sb = WK.tile([128, qn], F32, tag="Usb", name="U_sb",
                           padded_shape=[128, 512])
            nc.vector.tensor_copy(U_sb, U_ps)

            def epi_piece(c, ci=ci, q0=q0, qn=qn, ncg=ncg, U_sb=U_sb):
                cw = min(128, qn - c * 128)
                # last chunk: main loop is done, the S slots are free -- use
                # them so the exposed tail divide double-buffers and the
                # TP slot stays free for the conv chunks
                last = ci == len(CHUNKS) - 1
                TPt = (SP if last else TP).tile(
                    [128, 132], F32,
                    tag="S" if last else "T", name="TPt",
                    padded_shape=None if last else [128, 512])
                Ut = TPt[0:cw, 0:4]
                # transpose-and-sum the column-group partials
                for j in range(ncg):
                    nc.tensor.matmul(
                        Ut,
                        lhsT=U_sb[32 * j:32 * j + 4, c * 128:c * 128 + cw],
                        rhs=eye4x_sb[32 * j:32 * j + 4, :],
                        start=(j == 0), stop=(j == ncg - 1),
                        skip_group_check=True,
                    )
                r_sb = SM.tile([128, 1], F32, tag="r", name="r_sb")
                nc.vector.reciprocal(r_sb[0:cw, :], Ut[:, 3:4])
                Ft_sb = SM.tile([128, 4], F32, tag="Ft", name="Ft_sb")
                nc.vector.tensor_scalar_mul(Ft_sb[0:cw, :], Ut, r_sb[0:cw, :])
                Fb = TPt[0:4, 4:4 + cw]
                nc.tensor.matmul(Fb, lhsT=Ft_sb[0:cw, :],
                                 rhs=id128_sb[0:cw, 0:cw],
                                 start=True, stop=True)
                lo = q0 + c * 128
                nc.vector.tensor_copy(F_sb[:, lo:lo + cw], Fb[0:3, :])
                fp_copy(lo, lo + cw)
                pstate["q"] = lo + cw
                advance()

            pending_epi += [(lambda c=c, f=epi_piece: f(c))
                            for c in range((qn + 127) // 128)]
            if ci == 0:
                pending_epi.append(exchange_start)
            elif ci == 2:
                pending_epi.append(exchange_blend)

        while pending_epi:
            pending_epi.pop(0)()
        finalize()

    nc.compile()
    return nc


_CACHE: dict = {}


def _get_nc() -> bass.Bass:
    if "nc" not in _CACHE:
        _CACHE["nc"] = build_nc()
    return _CACHE["nc"]


def make_in_maps(input, conv_w, conv_b):
    input = np.ascontiguousarray(np.asarray(input, dtype=np.float32))
    conv_w = np.asarray(conv_w, dtype=np.float32)
    conv_b = np.asarray(conv_b, dtype=np.float32)
    x = input.reshape(B, C, N)

    # conv weights: wmat[3s+i, o] = conv_w[o, i, dy+1, dx+1] for shift s
    wmat = np.empty((27, C), np.float32)
    for s, (dy, dx) in enumerate(SHIFTS):
        wmat[3 * s:3 * s + 3, :] = conv_w[:, :, dy + 1, dx + 1].T  # [i, o]
    wmat = wmat.astype(ml_dtypes.bfloat16)
    eye4x = np.zeros((128, 4), np.float32)
    for j in range(4):
        eye4x[32 * j:32 * j + 4, :] = np.eye(4, dtype=np.float32)
    id128 = np.eye(128, dtype=np.float32)

    in_maps = []
    for b in range(B):
        xb = x[b]
        xk_h = xb.astype(ml_dtypes.bfloat16)
        v4 = np.ones((128, KB, 4), np.float32)
        v4[:, :, :3] = xb.reshape(C, KB, 128).transpose(2, 1, 0)
        v4 = v4.reshape(128, KB * 4).astype(ml_dtypes.bfloat16)
        for j in range(BANDS):
            r0 = j * BAND_ROWS
            band = input[b][:, r0:r0 + BAND_ROWS, :]       # [C, 24, 96]
            xqp = np.empty((C, QN), np.float32)
            xqp[:, 0:96] = band[:, 0, :]
            xqp[:, 96:192] = band[:, 23, :]
            xqp[:, 192:] = band[:, 1:23, :].reshape(C, 22 * 96)
            # exchange blend masks: [side, g, e, w] flattened to 1536.
            # side 0 fills F_p row 0 with neighbor (j-1)'s bottom edge
            # (e=1); side 1 fills row 25 with (j+1)'s top edge (e=0).
            ms = np.zeros((2, BANDS, 2, W), np.float32)
            if j > 0:
                ms[0, j - 1, 1, :] = 1.0
            if j < BANDS - 1:
                ms[1, j + 1, 0, :] = 1.0
            msel_h = np.broadcast_to(
                ms.reshape(1, 1536), (C, 1536)).copy()
            residb = np.zeros((C, BAND_ROWS, WP), np.float32)
            residb[:, :, 1:1 + W] = band + conv_b[:, None, None]
            in_maps.append({
                "xk": xk_h,
                "xq": xqp.astype(ml_dtypes.bfloat16),
                "v4": v4,
                "resid": residb.reshape(C, CONVN),
                "msel": msel_h,
                "wmat": wmat,
                "eye4x": eye4x,
                "id128": id128,
            })
    return in_maps


def run(input, conv_w, conv_b, trace=False, **spmd_kwargs):
    in_maps = make_in_maps(input, conv_w, conv_b)
    res = run_bass_kernel_spmd(_get_nc(), in_maps, list(range(2 * BANDS)),
                               trace=trace, **spmd_kwargs)
    out = np.empty((B, C, H, W), np.float32)
    for b in range(B):
        for j in range(BANDS):
            band = res.results[b * BANDS + j]["out"]
            out[b, :, j * BAND_ROWS:(j + 1) * BAND_ROWS, :] = (
                band.reshape(C, BAND_ROWS, W))
    return out, res


def kernel(input, conv_w, conv_b) -> np.ndarray:
    out, _ = run(input, conv_w, conv_b)
    return out
